# revision 27
# baseline (speedup 1.0000x reference)
"""8-core sharded BertGraphSelfAttention as a Bass/Tile kernel for Trainium2.

Sharding: data-parallel over batch b (16 -> 2 per core). Each core runs the
full two-branch attention on its 2 batches; host gathers the outputs.

Per-core pipeline (per batch b):
  QKV1   : tokens in (s,m) order, 108-row tiles, PE-transposed X ->
           Q1T/K1T (pre-scaled, biased, bf16) + V1 (+bias) resident in SBUF.
  branch1: s-triples packed to 108 partitions; per-(n,h) 36x36 score matmuls
           into one PSUM bank [108,144]; host-prescaled graph bias added on
           DVE; row softmax (fused exp+rowsum on ACT); PE-transposed probs;
           ctx matmuls -> X2 in SBUF via SBUF->SBUF permute DMA.
  QKV2   : 512-token supertiles -> Q2T/K2T SBUF bf16, V2 (+bias) -> DRAM.
  branch2: rel-k via per-(q,h) matmuls with host-built rkT[q] stationary ->
           RELT, folded into scores by PSUM transpose-accumulate; mask bias
           via K=1 matmul; row softmax with deferred 1/rowsum; ctx2T + rel-v
           batched per (q,h); final PE transpose + rowsum scale -> out.
"""

import math
import sys

import numpy as np

sys.path.insert(0, "/opt/trn_rl_repo")

import ml_dtypes

BF = ml_dtypes.bfloat16

H = 4
HD = 128
MAXREL = 16
B, M, SEQ, DIM = 16, 36, 128, 512
N_CORES = 8
BSH = B // N_CORES  # batches per core (2)
SC = 1.0 / math.sqrt(HD)

# branch1 s-grouping: 42 triples + one pair per seq of 128
S_GROUPS = [(3 * i, 3) for i in range(42)] + [(126, 2)]
N_TILES1 = len(S_GROUPS)  # 43 token tiles of 108/72 rows per batch


def _dist():
    r = np.arange(SEQ)
    return np.clip(r[None, :] - r[:, None], -MAXREL, MAXREL) + MAXREL


# ---------------------------------------------------------------------------
# device kernel builder
# ---------------------------------------------------------------------------

def build_kernel(debug=None, with_mask_bias=False, repeat=None):
    import concourse.bass as bass
    import concourse.mybir as mybir
    import concourse.tile as tile
    from concourse.masks import make_identity

    import bass_patches

    bass_patches.apply()

    fp32 = mybir.dt.float32
    bf16 = mybir.dt.bfloat16
    AF = mybir.ActivationFunctionType
    ALU = mybir.AluOpType
    AX = mybir.AxisListType

    nc = bass.Bass(trn_type="TRN2", target_bir_lowering=False, debug=False)

    # ---- DRAM I/O ----
    hs_d = nc.dram_tensor("hs", [BSH, M, SEQ, DIM], bf16, kind="ExternalInput").ap()
    sg_d = nc.dram_tensor("sg", [BSH, SEQ, H, M, M], fp32, kind="ExternalInput").ap()
    mb2_d = nc.dram_tensor("mb2", [BSH, M * SEQ], bf16, kind="ExternalInput").ap()
    w_names = ["Wq1", "Wk1", "Wv1", "Wq2", "Wk2", "Wv2"]
    w_d = {n: nc.dram_tensor(n, [DIM, DIM], bf16, kind="ExternalInput").ap()
           for n in w_names}
    bqk_d = nc.dram_tensor("bqk", [4, DIM], fp32, kind="ExternalInput").ap()
    bv_d = nc.dram_tensor("bvr", [2, 128, DIM], fp32, kind="ExternalInput").ap()
    rkT_d = nc.dram_tensor("rkT", [SEQ, HD, SEQ], bf16, kind="ExternalInput").ap()
    rv_d = nc.dram_tensor("rv", [SEQ, SEQ, HD], bf16, kind="ExternalInput").ap()
    out_d = nc.dram_tensor("out", [BSH, M, SEQ, DIM], fp32,
                           kind="ExternalOutput").ap()
    v2_d = nc.dram_tensor("v2scratch", [BSH, M, SEQ, DIM], bf16,
                          kind="Internal").ap()
    q2T_d = nc.dram_tensor("q2Tscratch", [BSH, H, 128, M * SEQ], bf16,
                           kind="Internal").ap()
    k2T_d = nc.dram_tensor("k2Tscratch", [BSH, H, 128, M * SEQ], bf16,
                           kind="Internal").ap()
    dbg = {}
    if debug:
        for name, shape in debug.items():
            dbg[name] = nc.dram_tensor(name, shape, fp32,
                                       kind="ExternalOutput").ap()

    NT = M * SEQ  # tokens per batch = 4608

    with tile.TileContext(nc, pool_alloc_mode="queue") as tc:
        with tc.tile_pool(name="consts", bufs=1) as cpool:
            ident = cpool.tile([128, 128], bf16)
            make_identity(nc, ident[:])
            identf = cpool.tile([128, 128], fp32)
            make_identity(nc, identf[:])
            ones1 = cpool.tile([1, 128], bf16)
            nc.vector.memset(ones1[:], 1.0)
            bqk_sb = cpool.tile([128, 4 * 4], fp32)  # [128, (mat, oc)]
            nc.sync.dma_start(
                bqk_sb[:].rearrange("p (m oc) -> p m oc", m=4),
                bqk_d.rearrange("m (oc p) -> p m oc", p=128))
            bv1_sb = cpool.tile([128, DIM], fp32)
            nc.sync.dma_start(bv1_sb[:], bv_d[0])
            bv2_sb = cpool.tile([128, DIM], fp32)
            nc.sync.dma_start(bv2_sb[:], bv_d[1])
            mb2_sb = None
            if with_mask_bias:
                mb2_sb = cpool.tile([1, BSH * M * SEQ], bf16)
                nc.sync.dma_start(
                    mb2_sb[:], mb2_d.rearrange("b t -> (b t)")[None, :])

            def body():
                for b in range(BSH):
                    _one_batch(tc, nc, b, hs_d, sg_d, mb2_sb, w_d, bqk_sb,
                               bv1_sb, bv2_sb, rkT_d, rv_d, v2_d, out_d,
                               q2T_d, k2T_d, ident, identf, ones1, dbg,
                               fp32, bf16, AF, ALU, AX, tile)

            if repeat is None:
                body()
            else:
                with tc.For_i(0, repeat, 1):
                    body()
    return nc


def _one_batch(tc, nc, b, hs_d, sg_d, mb2_sb, w_d, bqk_sb, bv1_sb, bv2_sb,
               rkT_d, rv_d, v2_d, out_d, q2T_d, k2T_d, ident, identf, ones1,
               dbg, fp32, bf16, AF, ALU, AX, tile):
    NT = M * SEQ

    # ---------------- phase 1: QKV1 + branch1 + QKV2 ----------------
    with tc.tile_pool(name="x2pool", bufs=1) as x2pool:
        # X2 accumulates branch1 output in (s-part, m, o) layout
        x2 = x2pool.tile([128, M * DIM], bf16, tag="x2")

        with tc.tile_pool(name="b1big", bufs=1) as big1:
            # resident: Q1T/K1T per head [128, NT] bf16, V1 pair-aligned
            q1T = [big1.tile([128, NT], bf16, name=f"q1T{h}", tag=f"q1T{h}") for h in range(H)]
            k1T = [big1.tile([128, NT], bf16, name=f"k1T{h}", tag=f"k1T{h}") for h in range(H)]
            v1 = big1.tile([128, (SEQ // 2) * DIM], bf16, tag="v1")

            _qkv1(tc, nc, b, hs_d, w_d, bqk_sb, bv1_sb, q1T, k1T, v1,
                  ident, fp32, bf16, AF, ALU, tile)

            if "q1T0" in dbg:
                for h in range(H):
                    _dump_bf16(tc, nc, dbg[f"q1T{h}"], q1T[h], fp32, bf16)
                    _dump_bf16(tc, nc, dbg[f"k1T{h}"], k1T[h], fp32, bf16)
                _dump_bf16(tc, nc, dbg["v1"], v1, fp32, bf16)
                return

            _branch1(tc, nc, b, sg_d, q1T, k1T, v1, bv1_sb, x2,
                     ident, fp32, bf16, AF, ALU, AX, tile)

        if "x2" in dbg:
            _dump_bf16(tc, nc, dbg["x2"], x2, fp32, bf16)
            return

        # QKV2 consumes x2; writes q2T/k2T (persistent pool) + V2 -> DRAM
        with tc.tile_pool(name="w2", bufs=1) as wpool2:
            w2 = {}
            for mat in ("Wq2", "Wk2", "Wv2"):
                t = wpool2.tile([128, 4 * DIM], bf16, name=mat, tag=mat)
                nc.sync.dma_start(
                    t[:].rearrange("p (ic o) -> p ic o", ic=4),
                    w_d[mat].rearrange("(ic p) o -> p ic o", p=128))
                w2[mat] = t

            _qkv2(tc, nc, b, x2, w2, bqk_sb, bv2_sb, q2T_d, k2T_d, v2_d,
                  ident, fp32, bf16, AF, ALU, tile)

    if "stop_after_qkv2" in dbg:
        return

    # ---------------- phase 2: branch2 ----------------
    _branch2(tc, nc, b, q2T_d, k2T_d, v2_d, mb2_sb, rkT_d, rv_d, out_d,
             ident, identf, ones1, fp32, bf16, AF, ALU, AX, tile, dbg)


def _dump_bf16(tc, nc, dst_ap, src_tile, fp32, bf16):
    """debug: bf16 SBUF tile -> fp32 DRAM (via fp32 SBUF staging)."""
    with tc.tile_pool(name="dump", bufs=2) as pool:
        p, f = src_tile.shape
        chunk = min(f, 2048)
        for c0 in range(0, f, chunk):
            cw = min(chunk, f - c0)
            stage = pool.tile([p, chunk], fp32, tag="stage")
            nc.any.tensor_copy(stage[:, :cw], src_tile[:, c0:c0 + cw])
            nc.sync.dma_start(dst_ap[:, c0:c0 + cw], stage[:, :cw])


# ---------------------------------------------------------------------------
# QKV1
# ---------------------------------------------------------------------------

def _qkv1(tc, nc, b, hs_d, w_d, bqk_sb, bv1_sb, q1T, k1T, v1,
          ident, fp32, bf16, AF, ALU, tile):
    # hs[b]: [M, SEQ, DIM] bf16; token rows (s, m)
    hsb = hs_d[b]  # [M, SEQ, DIM]
    with (
        tc.tile_pool(name="w1", bufs=1) as wpool,
        tc.tile_pool(name="x1", bufs=4) as xpool,
        tc.tile_pool(name="x1T", bufs=3) as xtpool,
        tc.tile_pool(name="ps_tr1", bufs=2, space="PSUM") as ptr,
        tc.tile_pool(name="ps_qk1", bufs=4, space="PSUM") as pqk,
        tc.tile_pool(name="ps_v1", bufs=2, space="PSUM") as pv,
        tc.tile_pool(name="vst1", bufs=4) as vstpool,
    ):
        w1 = {}
        for mat in ("Wq1", "Wk1", "Wv1"):
            t = wpool.tile([128, 4 * DIM], bf16, name=mat, tag=mat)
            nc.sync.dma_start(
                t[:].rearrange("p (ic o) -> p ic o", ic=4),
                w_d[mat].rearrange("(ic p) o -> p ic o", p=128))
            w1[mat] = t
        for g, (s0, ns) in enumerate(S_GROUPS):
            rows = ns * M
            # load X tile [rows, 512] (rows = (s, m))
            xt = xpool.tile([108, DIM], bf16, tag="x")
            for ds in range(ns):
                nc.sync.dma_start(xt[ds * M:(ds + 1) * M, :],
                                  hsb[:, s0 + ds, :])
            # transpose -> XT [4][128, rows]
            xT = xtpool.tile([128, 4 * 108], bf16, tag="xT")
            for ic in range(4):
                tp = ptr.tile([128, 108], bf16, tag="tr")
                nc.tensor.transpose(tp[:, :rows], xt[:rows, ic * 128:(ic + 1) * 128],
                                    ident[:rows, :rows])
                nc.any.tensor_copy(xT[:, ic * 108:ic * 108 + rows], tp[:, :rows])
            # Q1T/K1T (form B): out[o_chunk, rows]
            for mi, (mat, dstT, bcol) in enumerate(
                    (("Wq1", q1T, 0), ("Wk1", k1T, 1))):
                for oc in range(4):
                    acc = pqk.tile([128, 108], fp32, tag="qk")
                    for ic in range(4):
                        nc.tensor.matmul(
                            acc[:, :rows],
                            w1[mat][:, ic * DIM + oc * 128: ic * DIM + (oc + 1) * 128],
                            xT[:, ic * 108: ic * 108 + rows],
                            start=(ic == 0), stop=(ic == 3))
                    bias_ap = bqk_sb[:, bcol * 4 + oc: bcol * 4 + oc + 1]
                    if (oc + mi) % 2 == 0:
                        nc.vector.tensor_scalar(
                            dstT[oc][:, g * 108: g * 108 + rows],
                            acc[:, :rows], bias_ap, None, op0=ALU.add)
                    else:
                        nc.scalar.activation(
                            dstT[oc][:, g * 108: g * 108 + rows],
                            acc[:, :rows], AF.Identity, bias=bias_ap)
            # V1 (form A): out[rows, 512]; stage then pair-align into v1
            accv = pv.tile([108, DIM], fp32, tag="v")
            for ic in range(4):
                nc.tensor.matmul(
                    accv[:rows, :], xT[:, ic * 108: ic * 108 + rows],
                    w1["Wv1"][:, ic * DIM:(ic + 1) * DIM],
                    start=(ic == 0), stop=(ic == 3))
            vstage = vstpool.tile([108, DIM], bf16, tag="vstage")
            nc.vector.tensor_tensor(
                vstage[:rows, :], accv[:rows, :], bv1_sb[:rows, :], ALU.add)
            for ds in range(ns):
                s = s0 + ds
                nc.sync.dma_start(
                    v1[64 * (s % 2): 64 * (s % 2) + M,
                       (s // 2) * DIM:(s // 2 + 1) * DIM],
                    vstage[ds * M:(ds + 1) * M, :])


# ---------------------------------------------------------------------------
# branch1 attention
# ---------------------------------------------------------------------------

def _branch1(tc, nc, b, sg_d, q1T, k1T, v1, bv1_sb, x2,
             ident, fp32, bf16, AF, ALU, AX, tile):
    # s-pairs packed at partition offsets {0, 64} (PE 32-alignment rule)
    with (
        tc.tile_pool(name="b1sg", bufs=6) as sgpool,
        tc.tile_pool(name="b1s", bufs=6) as spool,
        tc.tile_pool(name="b1p", bufs=6) as probpool,
        tc.tile_pool(name="b1pt", bufs=6) as ptpool,
        tc.tile_pool(name="b1ctx", bufs=4) as ctxpool,
        tc.tile_pool(name="b1stat", bufs=8) as statpool,
        tc.tile_pool(name="ps_sc1", bufs=3, space="PSUM") as psc,
        tc.tile_pool(name="ps_pt1", bufs=3, space="PSUM") as ppt,
        tc.tile_pool(name="ps_cx1", bufs=2, space="PSUM") as pcx,
    ):
        for g2 in range(SEQ // 2):
            s0 = 2 * g2
            # scores psum [128, (h, k)]; s-pair at partition 0 / 64
            sc_ps = psc.tile([128, H * M], fp32, tag="sc")
            for h in range(H):
                for j in range(2):
                    col = (s0 + j) * M
                    nc.tensor.matmul(
                        sc_ps[64 * j:64 * j + M, h * M:(h + 1) * M],
                        q1T[h][:, col:col + M], k1T[h][:, col:col + M],
                        start=True, stop=True)
            # + graph bias (host pre-scaled by 1e4, fp32); zero garbage rows
            sg_sb = sgpool.tile([128, H * M], fp32, tag="sg")
            nc.vector.memset(sg_sb[32:64, :], 0.0)
            nc.vector.memset(sg_sb[96:128, :], 0.0)
            for j in range(2):
                nc.sync.dma_start(
                    sg_sb[64 * j:64 * j + M, :].rearrange(
                        "q (h k) -> q h k", h=H),
                    sg_d[b, s0 + j].rearrange("h q k -> q h k"))
            s_sb = spool.tile([128, H * M], fp32, tag="s")
            nc.vector.tensor_tensor(s_sb[:], sc_ps[:], sg_sb[:], ALU.add)
            # softmax over k segments (garbage rows bounded by max-sub);
            # per-h max subtract on POOL, single exp on ACT, sums on DVE
            nmax = statpool.tile([128, H], fp32, tag="nmax")
            nc.vector.tensor_reduce(
                nmax[:], s_sb[:].rearrange("p (h k) -> p h k", h=H),
                AX.X, ALU.max, negate=True)
            for h in range(H):
                nc.vector.tensor_scalar(
                    s_sb[:, h * M:(h + 1) * M], s_sb[:, h * M:(h + 1) * M],
                    nmax[:, h:h + 1], None, op0=ALU.add)
            prob = probpool.tile([128, H * M], bf16, tag="prob")
            nc.scalar.activation(prob[:], s_sb[:], AF.Exp)
            rsum = statpool.tile([128, H], fp32, tag="rsum")
            nc.vector.tensor_reduce(
                rsum[:], prob[:].rearrange("p (h k) -> p h k", h=H),
                AX.X, ALU.add)
            rinv = statpool.tile([128, H], fp32, tag="rinv")
            nc.vector.reciprocal(rinv[:], rsum[:])
            for h in range(H):
                nc.vector.tensor_scalar(
                    prob[:, h * M:(h + 1) * M],
                    prob[:, h * M:(h + 1) * M], rinv[:, h:h + 1], None,
                    op0=ALU.mult)
            # transpose probs per (h, j): [36q, 36k] -> [36k, 36q] at base 64j
            pT_ps = ppt.tile([128, H * M], bf16, tag="pt")
            for h in range(H):
                for j in range(2):
                    nc.tensor.transpose(
                        pT_ps[64 * j:64 * j + M, h * M:(h + 1) * M],
                        prob[64 * j:64 * j + M, h * M:(h + 1) * M],
                        ident[64 * j:64 * j + M, 64 * j:64 * j + M])
            pT = ptpool.tile([128, H * M], bf16, tag="pT")
            nc.any.tensor_copy(pT[:], pT_ps[:])
            # ctx: per (h, j) matmul -> psum [128, 512]
            cx = pcx.tile([128, DIM], fp32, tag="cx")
            for h in range(H):
                for j in range(2):
                    nc.tensor.matmul(
                        cx[64 * j:64 * j + M, h * HD:(h + 1) * HD],
                        pT[64 * j:64 * j + M, h * M:(h + 1) * M],
                        v1[64 * j:64 * j + M,
                           g2 * DIM + h * HD: g2 * DIM + (h + 1) * HD],
                        start=True, stop=True)
            # ctx -> x2 [s-part, (m, o)] via SBUF->SBUF permute DMA
            cs = ctxpool.tile([128, DIM], bf16, tag="cs")
            nc.any.tensor_copy(cs[:], cx[:])
            for j in range(2):
                nc.sync.dma_start(x2[s0 + j:s0 + j + 1, :],
                                  cs[64 * j:64 * j + M, :])


# ---------------------------------------------------------------------------
# QKV2
# ---------------------------------------------------------------------------

def _qkv2(tc, nc, b, x2, w2, bqk_sb, bv2_sb, q2T_d, k2T_d, v2_d,
          ident, fp32, bf16, AF, ALU, tile):
    x2v = x2[:].rearrange("s (m o) -> s m o", m=M)
    with (
        tc.tile_pool(name="x2T", bufs=2) as xtpool,
        tc.tile_pool(name="v2s", bufs=3) as v2pool,
        tc.tile_pool(name="ps_tr2", bufs=3, space="PSUM") as ptr,
        tc.tile_pool(name="ps_qk2", bufs=3, space="PSUM") as pqk,
        tc.tile_pool(name="ps_v2", bufs=2, space="PSUM") as pv,
        tc.tile_pool(name="qkst", bufs=6) as qkstpool,
    ):
        for st in range(M // 4):  # supertiles of 4 m-tiles (512 tokens)
            ms = [4 * st + i for i in range(4)]
            xT = [xtpool.tile([128, 512], bf16, name=f"xT{ic}", tag=f"xT{ic}") for ic in range(4)]
            for ic in range(4):
                for mi, m in enumerate(ms):
                    tp = ptr.tile([128, 128], bf16, tag="tr")
                    nc.tensor.transpose(
                        tp[:], x2v[:, m, ic * 128:(ic + 1) * 128], ident[:])
                    nc.any.tensor_copy(xT[ic][:, mi * 128:(mi + 1) * 128], tp[:])
            for mi2, (mat, dstT, bcol) in enumerate(
                    (("Wq2", q2T_d, 2), ("Wk2", k2T_d, 3))):
                for oc in range(4):
                    acc = pqk.tile([128, 512], fp32, tag="qk")
                    for ic in range(4):
                        nc.tensor.matmul(
                            acc[:],
                            w2[mat][:, ic * DIM + oc * 128: ic * DIM + (oc + 1) * 128],
                            xT[ic][:], start=(ic == 0), stop=(ic == 3))
                    qkst = qkstpool.tile([128, 512], bf16, tag="qkst")
                    bias_ap = bqk_sb[:, bcol * 4 + oc: bcol * 4 + oc + 1]
                    if (oc + mi2) % 2 == 0:
                        nc.vector.tensor_scalar(
                            qkst[:], acc[:], bias_ap, None, op0=ALU.add)
                    else:
                        nc.scalar.activation(
                            qkst[:], acc[:], AF.Identity, bias=bias_ap)
                    nc.sync.dma_start(
                        dstT[b, oc][:, st * 512:(st + 1) * 512], qkst[:])
            # V2 (form A) per m-tile
            for mi, m in enumerate(ms):
                accv = pv.tile([128, DIM], fp32, tag="v")
                for ic in range(4):
                    nc.tensor.matmul(
                        accv[:], xT[ic][:, mi * 128:(mi + 1) * 128],
                        w2["Wv2"][:, ic * DIM:(ic + 1) * DIM],
                        start=(ic == 0), stop=(ic == 3))
                vs = v2pool.tile([128, DIM], bf16, tag="vs")
                nc.vector.tensor_tensor(vs[:], accv[:], bv2_sb[:], ALU.add)
                nc.sync.dma_start(
                    v2_d[b, m].rearrange("s o -> s o"), vs[:])


# ---------------------------------------------------------------------------
# branch2 attention
# ---------------------------------------------------------------------------

def _branch2(tc, nc, b, q2T_d, k2T_d, v2_d, mb2_sb, rkT_d, rv_d, out_d,
             ident, identf, ones1, fp32, bf16, AF, ALU, AX, tile, dbg):
    HALF = M // 2  # 18
    NT = M * SEQ
    with (
        tc.tile_pool(name="b2big", bufs=1) as big,
        tc.tile_pool(name="b2stat", bufs=1) as statpool,
    ):
        q2T = [big.tile([128, NT], bf16, name=f"q2Ts{h}", tag=f"q2Ts{h}")
               for h in range(H)]
        k2T = [big.tile([128, NT], bf16, name=f"k2Ts{h}", tag=f"k2Ts{h}")
               for h in range(H)]
        for h in range(H):
            nc.sync.dma_start(q2T[h][:], q2T_d[b, h])
            nc.sync.dma_start(k2T[h][:], k2T_d[b, h])
        p2T = big.tile([128, M * H * 128], bf16, tag="p2T")
        c2T = [big.tile([128, M * 128], bf16, name=f"c2T{h}", tag=f"c2T{h}") for h in range(H)]
        rs_all = statpool.tile([128, M * H], fp32, tag="rs")

        for half in range(2):
            n0 = half * HALF
            # ---- rel-k pass for this half ----
            with tc.tile_pool(name="relk", bufs=1) as relpool:
                relT = relpool.tile([128, HALF * H * 128], fp32, tag="relT")
                with (
                    tc.tile_pool(name="rkt", bufs=6) as rktpool,
                    tc.tile_pool(name="ps_rel", bufs=4, space="PSUM") as prel,
                ):
                    for q in range(SEQ):
                        rkt = rktpool.tile([128, 128], bf16, tag="rkt")
                        nc.sync.dma_start(rkt[:], rkT_d[q])
                        rp = prel.tile([128, H * HALF], fp32, tag="rp")
                        for h in range(H):
                            nc.tensor.matmul(
                                rp[:, h * HALF:(h + 1) * HALF], rkt[:],
                                q2T[h][:].rearrange("d (n q) -> d n q", q=SEQ)
                                [:, n0:n0 + HALF, q],
                                start=True, stop=True)
                        # scatter [k, (h, nn)] -> relT [k, (nn, h, q)]
                        nc.any.tensor_copy(
                            relT[:].rearrange("k (n h q) -> k n h q",
                                              h=H, q=SEQ)[:, :, :, q],
                            rp[:].rearrange("k (h n) -> k n h", h=H))

                # ---- main pass for this half ----
                with (
                    tc.tile_pool(name="b2s", bufs=3) as spool,
                    tc.tile_pool(name="b2e", bufs=6) as epool,
                    tc.tile_pool(name="b2v", bufs=4) as vpool,
                    tc.tile_pool(name="b2nm", bufs=8) as nmpool,
                    tc.tile_pool(name="ps_sc2", bufs=3, space="PSUM") as psc,
                    tc.tile_pool(name="ps_pt2", bufs=2, space="PSUM") as ppt,
                    tc.tile_pool(name="ps_cx2", bufs=2, space="PSUM") as pcx,
                ):
                    for nn in range(HALF):
                        n = n0 + nn
                        sc_ps = psc.tile([128, H * 128], fp32, tag="sc")
                        for h in range(H):
                            sl = sc_ps[:, h * 128:(h + 1) * 128]
                            nc.tensor.matmul(
                                sl, q2T[h][:, n * 128:(n + 1) * 128],
                                k2T[h][:, n * 128:(n + 1) * 128],
                                start=True, stop=False)
                            if mb2_sb is not None:
                                nc.tensor.matmul(
                                    sl, ones1[:],
                                    mb2_sb[:, (b * M + n) * 128:
                                           (b * M + n + 1) * 128],
                                    start=False, stop=False)
                            nc.tensor.matmul(
                                sl,
                                relT[:, (nn * H + h) * 128:(nn * H + h + 1) * 128],
                                identf[:], is_transpose=True,
                                start=False, stop=True)
                        s_sb = spool.tile([128, H * 128], fp32, tag="s")
                        nc.any.tensor_copy(s_sb[:], sc_ps[:])
                        nmax = nmpool.tile([128, H], fp32, tag="nmax")
                        nc.vector.tensor_reduce(
                            nmax[:], s_sb[:].rearrange("p (h k) -> p h k", h=H),
                            AX.X, ALU.max, negate=True)
                        e_sb = epool.tile([128, H * 128], bf16, tag="e")
                        for h in range(H):
                            nc.scalar.activation(
                                e_sb[:, h * 128:(h + 1) * 128],
                                s_sb[:, h * 128:(h + 1) * 128], AF.Exp,
                                bias=nmax[:, h:h + 1],
                                accum_out=rs_all[:, n * H + h:n * H + h + 1])
                        # transpose E per h -> p2T; ctx main matmuls
                        v2t = vpool.tile([128, DIM], bf16, tag="v2t")
                        nc.sync.dma_start(v2t[:], v2_d[b, n])
                        tp = ppt.tile([128, 4 * 128], bf16, tag="pt")
                        cxp = pcx.tile([128, 4 * 128], fp32, tag="cx")
                        for h in range(H):
                            nc.tensor.transpose(
                                tp[:, h * 128:(h + 1) * 128],
                                e_sb[:, h * 128:(h + 1) * 128], ident[:])
                            pslice = p2T[:, (n * H + h) * 128:(n * H + h + 1) * 128]
                            nc.any.tensor_copy(pslice, tp[:, h * 128:(h + 1) * 128])
                            nc.tensor.matmul(
                                cxp[:, h * 128:(h + 1) * 128],
                                v2t[:, h * HD:(h + 1) * HD], pslice,
                                start=True, stop=True)
                            nc.any.tensor_copy(
                                c2T[h][:, n * 128:(n + 1) * 128],
                                cxp[:, h * 128:(h + 1) * 128])

        # ---- rel-v pass (full batch) ----
        with (
            tc.tile_pool(name="rvt", bufs=6) as rvtpool,
            tc.tile_pool(name="ps_rv", bufs=4, space="PSUM") as prv,
        ):
            p2Tv = p2T[:].rearrange("k (n h q) -> k n h q", h=H, q=SEQ)
            for q in range(SEQ):
                rvt = rvtpool.tile([128, 128], bf16, tag="rvt")
                nc.sync.dma_start(rvt[:], rv_d[q])
                for h in range(H):
                    rp = prv.tile([128, M], fp32, tag="rp")
                    nc.tensor.matmul(rp[:], rvt[:], p2Tv[:, :, h, q],
                                     start=True, stop=True)
                    dst = c2T[h][:].rearrange("d (n q) -> d n q", q=SEQ)[:, :, q]
                    nc.vector.tensor_tensor(dst, rp[:], dst, ALU.add)

        # ---- output: transpose + 1/rowsum ----
        with (
            tc.tile_pool(name="ostat", bufs=1) as ostat,
            tc.tile_pool(name="osb", bufs=3) as opool,
            tc.tile_pool(name="ps_out", bufs=2, space="PSUM") as pout,
        ):
            rinv = ostat.tile([128, M * H], fp32, tag="rinv")
            nc.vector.reciprocal(rinv[:], rs_all[:])
            for n in range(M):
                op = pout.tile([128, DIM], bf16, tag="op")
                for h in range(H):
                    nc.tensor.transpose(
                        op[:, h * HD:(h + 1) * HD],
                        c2T[h][:, n * 128:(n + 1) * 128], ident[:])
                osb = opool.tile([128, DIM], fp32, tag="osb")
                for h in range(H):
                    nc.vector.tensor_scalar_mul(
                        osb[:, h * HD:(h + 1) * HD], op[:, h * HD:(h + 1) * HD],
                        rinv[:, n * H + h:n * H + h + 1])
                nc.sync.dma_start(out_d[b, n], osb[:])


# ---------------------------------------------------------------------------
# host side
# ---------------------------------------------------------------------------

_CACHE = {}


def _host_prep(hidden_states, attention_mask, sim_graph,
               Wq_sim, bq_sim, Wk_sim, bk_sim, Wv_sim, bv_sim,
               Wq_seq, bq_seq, Wk_seq, bk_seq, Wv_seq, bv_seq,
               rel_k, rel_v):
    """Build the 8 per-core input maps."""
    f32 = np.float32
    hs = np.asarray(hidden_states, f32)
    mask = np.asarray(attention_mask, f32)
    sg = np.asarray(sim_graph, f32)

    # branch1 graph bias, host-folded mask, pre-scaled by 1e4
    # sg_eff = where(mask_sim==0, 0, sg); bias = 1e4*sg_eff  (const dropped)
    mask_sim = mask.transpose(0, 2, 1).reshape(B * SEQ, M)  # [n, k]
    sg_eff = np.where(mask_sim[:, None, None, :] == 0, 0.0, sg) * 1e4
    sg_eff = sg_eff.reshape(B, SEQ, H, M, M).astype(f32)

    # branch2 mask bias rows
    mb2 = ((1.0 - mask.reshape(B * M, SEQ)) * -10000.0).astype(BF)
    mb2 = mb2.reshape(B, M * SEQ)

    d = _dist()
    rk_full = np.asarray(rel_k, f32)[d]          # [q, k, hd]
    rv_full = np.asarray(rel_v, f32)[d]          # [q, k, hd]
    rkT = np.ascontiguousarray(rk_full.transpose(0, 2, 1)).astype(BF)
    rv_b = np.ascontiguousarray(rv_full).astype(BF)

    ws = {"Wq1": np.asarray(Wq_sim, f32) * SC, "Wk1": Wk_sim,
          "Wv1": Wv_sim, "Wq2": np.asarray(Wq_seq, f32) * SC,
          "Wk2": Wk_seq, "Wv2": Wv_seq}
    ws = {k: np.asarray(v, f32).astype(BF) for k, v in ws.items()}
    bqk = np.stack([np.asarray(bq_sim, f32) * SC, np.asarray(bk_sim, f32),
                    np.asarray(bq_seq, f32) * SC, np.asarray(bk_seq, f32)])
    bv = np.stack([np.broadcast_to(np.asarray(bv_sim, f32), (128, DIM)),
                   np.broadcast_to(np.asarray(bv_seq, f32), (128, DIM))])
    bv = np.ascontiguousarray(bv)

    hs_bf = hs.astype(BF)
    in_maps = []
    for c in range(N_CORES):
        bs = slice(c * BSH, (c + 1) * BSH)
        in_maps.append(dict(
            hs=np.ascontiguousarray(hs_bf[bs]),
            sg=np.ascontiguousarray(sg_eff[bs]),
            mb2=np.ascontiguousarray(mb2[bs]),
            bqk=bqk, bvr=bv, rkT=rkT, rv=rv_b,
            **ws))
    return in_maps


def kernel(hidden_states, attention_mask, sim_graph,
           Wq_sim, bq_sim, Wk_sim, bk_sim, Wv_sim, bv_sim,
           Wq_seq, bq_seq, Wk_seq, bk_seq, Wv_seq, bv_seq,
           rel_k, rel_v, b=None, m=None, seq=None, dim=None, **_):
    from concourse.bass_utils import run_bass_kernel_spmd

    in_maps = _host_prep(hidden_states, attention_mask, sim_graph,
                         Wq_sim, bq_sim, Wk_sim, bk_sim, Wv_sim, bv_sim,
                         Wq_seq, bq_seq, Wk_seq, bk_seq, Wv_seq, bv_seq,
                         rel_k, rel_v)
    masked = not bool(np.all(np.asarray(attention_mask) == 1.0))
    key = ("nc", masked)
    if key not in _CACHE:
        _CACHE[key] = build_kernel(with_mask_bias=masked)
    res = run_bass_kernel_spmd(_CACHE[key], in_maps,
                               core_ids=list(range(N_CORES)))
    outs = [r["out"] for r in res.results]
    return np.concatenate(outs, axis=0)


# revision 29
# speedup vs baseline: 1.3877x; 1.3877x over previous
"""8-core sharded BertGraphSelfAttention as a Bass/Tile kernel for Trainium2.

Sharding: data-parallel over batch b (16 -> 2 per core). Each core runs the
full two-branch attention on its 2 batches; host gathers the outputs.

Per-core pipeline (per batch b):
  QKV1   : tokens in (s,m) order, 108-row tiles, PE-transposed X ->
           Q1T/K1T (pre-scaled, biased, bf16) + V1 (+bias) resident in SBUF.
  branch1: s-triples packed to 108 partitions; per-(n,h) 36x36 score matmuls
           into one PSUM bank [108,144]; host-prescaled graph bias added on
           DVE; row softmax (fused exp+rowsum on ACT); PE-transposed probs;
           ctx matmuls -> X2 in SBUF via SBUF->SBUF permute DMA.
  QKV2   : 512-token supertiles -> Q2T/K2T SBUF bf16, V2 (+bias) -> DRAM.
  branch2: rel-k via per-(q,h) matmuls with host-built rkT[q] stationary ->
           RELT, folded into scores by PSUM transpose-accumulate; mask bias
           via K=1 matmul; row softmax with deferred 1/rowsum; ctx2T + rel-v
           batched per (q,h); final PE transpose + rowsum scale -> out.
"""

import math
import sys

import numpy as np

sys.path.insert(0, "/opt/trn_rl_repo")

import ml_dtypes

BF = ml_dtypes.bfloat16

H = 4
HD = 128
MAXREL = 16
B, M, SEQ, DIM = 16, 36, 128, 512
N_CORES = 8
BSH = B // N_CORES  # batches per core (2)
SC = 1.0 / math.sqrt(HD)

# branch1 s-grouping: 42 triples + one pair per seq of 128
S_GROUPS = [(3 * i, 3) for i in range(42)] + [(126, 2)]
N_TILES1 = len(S_GROUPS)  # 43 token tiles of 108/72 rows per batch


def _dist():
    r = np.arange(SEQ)
    return np.clip(r[None, :] - r[:, None], -MAXREL, MAXREL) + MAXREL


# ---------------------------------------------------------------------------
# walrus workaround (inlined): this container's walrus encodes at most ONE
# sync-wait per instruction. Tile attaches several; before serializing we
# move extra waits onto single-wait NOPs inserted before the instruction on
# the same engine (identical blocking semantics).
# ---------------------------------------------------------------------------

def _apply_bass_patches():
    import concourse.bass as bass
    import concourse.mybir as mybir

    if getattr(bass.Bass, "_ant_wait_split_done", False):
        return

    def _split_multi_waits(m):
        ctr = [0]
        for fn in m.functions:
            for blk in fn.blocks:
                insts = blk.instructions
                if not any(i.sync_info and i.sync_info.on_wait
                           and len(i.sync_info.on_wait) > 1 for i in insts):
                    continue
                new = []
                for inst in insts:
                    si = inst.sync_info
                    if si is not None and si.on_wait and len(si.on_wait) > 1:
                        waits = list(si.on_wait)
                        for w in waits[:-1]:
                            ctr[0] += 1
                            nop = mybir.InstNoOp(
                                name=f"I-waitsplit-{ctr[0]}", ins=[], outs=[])
                            nop.engine = inst.engine
                            nop.sync_info = mybir.SyncInfo(
                                on_wait=[w], on_update=[])
                            new.append(nop)
                        si.on_wait = waits[-1:]
                    new.append(inst)
                blk.instructions[:] = new

    orig = bass.Bass.to_json_bytes

    def patched(self, *args, **kwargs):
        _split_multi_waits(self.m)
        return orig(self, *args, **kwargs)

    bass.Bass.to_json_bytes = patched
    bass.Bass._ant_wait_split_done = True


# ---------------------------------------------------------------------------
# device kernel builder
# ---------------------------------------------------------------------------

def build_kernel(debug=None, with_mask_bias=False, repeat=None):
    import concourse.bass as bass
    import concourse.mybir as mybir
    import concourse.tile as tile
    from concourse.masks import make_identity

    _apply_bass_patches()

    fp32 = mybir.dt.float32
    bf16 = mybir.dt.bfloat16
    AF = mybir.ActivationFunctionType
    ALU = mybir.AluOpType
    AX = mybir.AxisListType

    nc = bass.Bass(trn_type="TRN2", target_bir_lowering=False, debug=False)

    # ---- DRAM I/O ----
    hs_d = nc.dram_tensor("hs", [BSH, M, SEQ, DIM], bf16, kind="ExternalInput").ap()
    sg_d = nc.dram_tensor("sg", [BSH, SEQ, H, M, M], fp32, kind="ExternalInput").ap()
    mb2_d = nc.dram_tensor("mb2", [BSH, M * SEQ], bf16, kind="ExternalInput").ap()
    w_names = ["Wq1", "Wk1", "Wv1", "Wq2", "Wk2", "Wv2"]
    w_d = {n: nc.dram_tensor(n, [DIM, DIM], bf16, kind="ExternalInput").ap()
           for n in w_names}
    bqk_d = nc.dram_tensor("bqk", [4, DIM], fp32, kind="ExternalInput").ap()
    bv_d = nc.dram_tensor("bvr", [2, 128, DIM], fp32, kind="ExternalInput").ap()
    rkT_d = nc.dram_tensor("rkT", [SEQ, HD, SEQ], bf16, kind="ExternalInput").ap()
    rv_d = nc.dram_tensor("rv", [SEQ, SEQ, HD], bf16, kind="ExternalInput").ap()
    out_d = nc.dram_tensor("out", [BSH, M, SEQ, DIM], fp32,
                           kind="ExternalOutput").ap()
    v2_d = nc.dram_tensor("v2scratch", [BSH, M, SEQ, DIM], bf16,
                          kind="Internal").ap()
    q2T_d = nc.dram_tensor("q2Tscratch", [BSH, H, 128, M * SEQ], bf16,
                           kind="Internal").ap()
    k2T_d = nc.dram_tensor("k2Tscratch", [BSH, H, 128, M * SEQ], bf16,
                           kind="Internal").ap()
    dbg = {}
    if debug:
        for name, shape in debug.items():
            dbg[name] = nc.dram_tensor(name, shape, fp32,
                                       kind="ExternalOutput").ap()

    NT = M * SEQ  # tokens per batch = 4608

    with tile.TileContext(nc) as tc:
        with tc.tile_pool(name="consts", bufs=1) as cpool:
            ident = cpool.tile([128, 128], bf16)
            make_identity(nc, ident[:])
            identf = cpool.tile([128, 128], fp32)
            make_identity(nc, identf[:])
            ones1 = cpool.tile([1, 128], bf16)
            nc.vector.memset(ones1[:], 1.0)
            bqk_sb = cpool.tile([128, 4 * 4], fp32)  # [128, (mat, oc)]
            nc.sync.dma_start(
                bqk_sb[:].rearrange("p (m oc) -> p m oc", m=4),
                bqk_d.rearrange("m (oc p) -> p m oc", p=128))
            bv1_sb = cpool.tile([128, DIM], fp32)
            nc.sync.dma_start(bv1_sb[:], bv_d[0])
            bv2_sb = cpool.tile([128, DIM], fp32)
            nc.sync.dma_start(bv2_sb[:], bv_d[1])
            mb2_sb = None
            if with_mask_bias:
                mb2_sb = cpool.tile([1, BSH * M * SEQ], bf16)
                nc.sync.dma_start(
                    mb2_sb[:], mb2_d.rearrange("b t -> (b t)")[None, :])

            def body():
                for b in range(BSH):
                    _one_batch(tc, nc, b, hs_d, sg_d, mb2_sb, w_d, bqk_sb,
                               bv1_sb, bv2_sb, rkT_d, rv_d, v2_d, out_d,
                               q2T_d, k2T_d, ident, identf, ones1, dbg,
                               fp32, bf16, AF, ALU, AX, tile)

            if repeat is None:
                body()
            else:
                with tc.For_i(0, repeat, 1):
                    body()
    return nc


def _one_batch(tc, nc, b, hs_d, sg_d, mb2_sb, w_d, bqk_sb, bv1_sb, bv2_sb,
               rkT_d, rv_d, v2_d, out_d, q2T_d, k2T_d, ident, identf, ones1,
               dbg, fp32, bf16, AF, ALU, AX, tile):
    NT = M * SEQ

    # ---------------- phase 1: QKV1 + branch1 + QKV2 ----------------
    with tc.tile_pool(name="x2pool", bufs=1) as x2pool:
        # X2 accumulates branch1 output in (s-part, m, o) layout
        x2 = x2pool.tile([128, M * DIM], bf16, tag="x2")

        with tc.tile_pool(name="b1big", bufs=1) as big1:
            # resident: Q1T/K1T per head [128, NT] bf16, V1 pair-aligned
            q1T = [big1.tile([128, NT], bf16, name=f"q1T{h}", tag=f"q1T{h}") for h in range(H)]
            k1T = [big1.tile([128, NT], bf16, name=f"k1T{h}", tag=f"k1T{h}") for h in range(H)]
            v1 = big1.tile([128, (SEQ // 2) * DIM], bf16, tag="v1")

            _qkv1(tc, nc, b, hs_d, w_d, bqk_sb, bv1_sb, q1T, k1T, v1,
                  ident, fp32, bf16, AF, ALU, tile)

            if "q1T0" in dbg:
                for h in range(H):
                    _dump_bf16(tc, nc, dbg[f"q1T{h}"], q1T[h], fp32, bf16)
                    _dump_bf16(tc, nc, dbg[f"k1T{h}"], k1T[h], fp32, bf16)
                _dump_bf16(tc, nc, dbg["v1"], v1, fp32, bf16)
                return

            _branch1(tc, nc, b, sg_d, q1T, k1T, v1, bv1_sb, x2,
                     ident, fp32, bf16, AF, ALU, AX, tile)

        if "x2" in dbg:
            _dump_bf16(tc, nc, dbg["x2"], x2, fp32, bf16)
            return

        # QKV2 consumes x2; writes q2T/k2T (persistent pool) + V2 -> DRAM
        with tc.tile_pool(name="w2", bufs=1) as wpool2:
            w2 = {}
            for mat in ("Wq2", "Wk2", "Wv2"):
                t = wpool2.tile([128, 4 * DIM], bf16, name=mat, tag=mat)
                nc.sync.dma_start(
                    t[:].rearrange("p (ic o) -> p ic o", ic=4),
                    w_d[mat].rearrange("(ic p) o -> p ic o", p=128))
                w2[mat] = t

            _qkv2(tc, nc, b, x2, w2, bqk_sb, bv2_sb, q2T_d, k2T_d, v2_d,
                  ident, fp32, bf16, AF, ALU, tile)

    if "stop_after_qkv2" in dbg:
        return

    # ---------------- phase 2: branch2 ----------------
    _branch2(tc, nc, b, q2T_d, k2T_d, v2_d, mb2_sb, rkT_d, rv_d, out_d,
             ident, identf, ones1, fp32, bf16, AF, ALU, AX, tile, dbg)


def _dump_bf16(tc, nc, dst_ap, src_tile, fp32, bf16):
    """debug: bf16 SBUF tile -> fp32 DRAM (via fp32 SBUF staging)."""
    with tc.tile_pool(name="dump", bufs=2) as pool:
        p, f = src_tile.shape
        chunk = min(f, 2048)
        for c0 in range(0, f, chunk):
            cw = min(chunk, f - c0)
            stage = pool.tile([p, chunk], fp32, tag="stage")
            nc.any.tensor_copy(stage[:, :cw], src_tile[:, c0:c0 + cw])
            nc.sync.dma_start(dst_ap[:, c0:c0 + cw], stage[:, :cw])


# ---------------------------------------------------------------------------
# QKV1
# ---------------------------------------------------------------------------

def _qkv1(tc, nc, b, hs_d, w_d, bqk_sb, bv1_sb, q1T, k1T, v1,
          ident, fp32, bf16, AF, ALU, tile):
    # hs[b]: [M, SEQ, DIM] bf16; token rows (s, m)
    hsb = hs_d[b]  # [M, SEQ, DIM]
    with (
        tc.tile_pool(name="w1", bufs=1) as wpool,
        tc.tile_pool(name="x1", bufs=4) as xpool,
        tc.tile_pool(name="x1T", bufs=3) as xtpool,
        tc.tile_pool(name="ps_tr1", bufs=2, space="PSUM") as ptr,
        tc.tile_pool(name="ps_qk1", bufs=4, space="PSUM") as pqk,
        tc.tile_pool(name="ps_v1", bufs=2, space="PSUM") as pv,
        tc.tile_pool(name="vst1", bufs=4) as vstpool,
    ):
        w1 = {}
        for mat in ("Wq1", "Wk1", "Wv1"):
            t = wpool.tile([128, 4 * DIM], bf16, name=mat, tag=mat)
            nc.sync.dma_start(
                t[:].rearrange("p (ic o) -> p ic o", ic=4),
                w_d[mat].rearrange("(ic p) o -> p ic o", p=128))
            w1[mat] = t
        for g, (s0, ns) in enumerate(S_GROUPS):
            rows = ns * M
            # load X tile [rows, 512] (rows = (s, m))
            xt = xpool.tile([108, DIM], bf16, tag="x")
            for ds in range(ns):
                nc.sync.dma_start(xt[ds * M:(ds + 1) * M, :],
                                  hsb[:, s0 + ds, :])
            # transpose -> XT [4][128, rows]
            xT = xtpool.tile([128, 4 * 108], bf16, tag="xT")
            for ic in range(4):
                tp = ptr.tile([128, 108], bf16, tag="tr")
                nc.tensor.transpose(tp[:, :rows], xt[:rows, ic * 128:(ic + 1) * 128],
                                    ident[:rows, :rows])
                nc.any.tensor_copy(xT[:, ic * 108:ic * 108 + rows], tp[:, :rows])
            # Q1T/K1T (form B): out[o_chunk, rows]
            for mi, (mat, dstT, bcol) in enumerate(
                    (("Wq1", q1T, 0), ("Wk1", k1T, 1))):
                for oc in range(4):
                    acc = pqk.tile([128, 108], fp32, tag="qk")
                    for ic in range(4):
                        nc.tensor.matmul(
                            acc[:, :rows],
                            w1[mat][:, ic * DIM + oc * 128: ic * DIM + (oc + 1) * 128],
                            xT[:, ic * 108: ic * 108 + rows],
                            start=(ic == 0), stop=(ic == 3))
                    bias_ap = bqk_sb[:, bcol * 4 + oc: bcol * 4 + oc + 1]
                    if (oc + mi) % 2 == 0:
                        nc.vector.tensor_scalar(
                            dstT[oc][:, g * 108: g * 108 + rows],
                            acc[:, :rows], bias_ap, None, op0=ALU.add)
                    else:
                        nc.scalar.activation(
                            dstT[oc][:, g * 108: g * 108 + rows],
                            acc[:, :rows], AF.Identity, bias=bias_ap)
            # V1 (form A): out[rows, 512]; stage then pair-align into v1
            accv = pv.tile([108, DIM], fp32, tag="v")
            for ic in range(4):
                nc.tensor.matmul(
                    accv[:rows, :], xT[:, ic * 108: ic * 108 + rows],
                    w1["Wv1"][:, ic * DIM:(ic + 1) * DIM],
                    start=(ic == 0), stop=(ic == 3))
            vstage = vstpool.tile([108, DIM], bf16, tag="vstage")
            nc.vector.tensor_tensor(
                vstage[:rows, :], accv[:rows, :], bv1_sb[:rows, :], ALU.add)
            for ds in range(ns):
                s = s0 + ds
                nc.sync.dma_start(
                    v1[64 * (s % 2): 64 * (s % 2) + M,
                       (s // 2) * DIM:(s // 2 + 1) * DIM],
                    vstage[ds * M:(ds + 1) * M, :])


# ---------------------------------------------------------------------------
# branch1 attention
# ---------------------------------------------------------------------------

def _branch1(tc, nc, b, sg_d, q1T, k1T, v1, bv1_sb, x2,
             ident, fp32, bf16, AF, ALU, AX, tile):
    # s-pairs packed at partition offsets {0, 64} (PE 32-alignment rule)
    with (
        tc.tile_pool(name="b1sg", bufs=6) as sgpool,
        tc.tile_pool(name="b1s", bufs=6) as spool,
        tc.tile_pool(name="b1p", bufs=6) as probpool,
        tc.tile_pool(name="b1pt", bufs=6) as ptpool,
        tc.tile_pool(name="b1ctx", bufs=4) as ctxpool,
        tc.tile_pool(name="b1stat", bufs=8) as statpool,
        tc.tile_pool(name="ps_sc1", bufs=3, space="PSUM") as psc,
        tc.tile_pool(name="ps_pt1", bufs=3, space="PSUM") as ppt,
        tc.tile_pool(name="ps_cx1", bufs=2, space="PSUM") as pcx,
    ):
        for g2 in range(SEQ // 2):
            s0 = 2 * g2
            # scores psum [128, (h, k)]; s-pair at partition 0 / 64
            sc_ps = psc.tile([128, H * M], fp32, tag="sc")
            for h in range(H):
                for j in range(2):
                    col = (s0 + j) * M
                    nc.tensor.matmul(
                        sc_ps[64 * j:64 * j + M, h * M:(h + 1) * M],
                        q1T[h][:, col:col + M], k1T[h][:, col:col + M],
                        start=True, stop=True)
            # + graph bias (host pre-scaled by 1e4, fp32); zero garbage rows
            sg_sb = sgpool.tile([128, H * M], fp32, tag="sg")
            nc.vector.memset(sg_sb[32:64, :], 0.0)
            nc.vector.memset(sg_sb[96:128, :], 0.0)
            for j in range(2):
                nc.sync.dma_start(
                    sg_sb[64 * j:64 * j + M, :].rearrange(
                        "q (h k) -> q h k", h=H),
                    sg_d[b, s0 + j].rearrange("h q k -> q h k"))
            s_sb = spool.tile([128, H * M], fp32, tag="s")
            nc.vector.tensor_tensor(s_sb[:], sc_ps[:], sg_sb[:], ALU.add)
            # softmax over k segments (garbage rows bounded by max-sub);
            # per-h max subtract on POOL, single exp on ACT, sums on DVE
            nmax = statpool.tile([128, H], fp32, tag="nmax")
            nc.vector.tensor_reduce(
                nmax[:], s_sb[:].rearrange("p (h k) -> p h k", h=H),
                AX.X, ALU.max, negate=True)
            for h in range(H):
                nc.vector.tensor_scalar(
                    s_sb[:, h * M:(h + 1) * M], s_sb[:, h * M:(h + 1) * M],
                    nmax[:, h:h + 1], None, op0=ALU.add)
            prob = probpool.tile([128, H * M], bf16, tag="prob")
            nc.scalar.activation(prob[:], s_sb[:], AF.Exp)
            rsum = statpool.tile([128, H], fp32, tag="rsum")
            nc.vector.tensor_reduce(
                rsum[:], prob[:].rearrange("p (h k) -> p h k", h=H),
                AX.X, ALU.add)
            rinv = statpool.tile([128, H], fp32, tag="rinv")
            nc.vector.reciprocal(rinv[:], rsum[:])
            for h in range(H):
                nc.vector.tensor_scalar(
                    prob[:, h * M:(h + 1) * M],
                    prob[:, h * M:(h + 1) * M], rinv[:, h:h + 1], None,
                    op0=ALU.mult)
            # transpose probs per (h, j): [36q, 36k] -> [36k, 36q] at base 64j
            pT_ps = ppt.tile([128, H * M], bf16, tag="pt")
            for h in range(H):
                for j in range(2):
                    nc.tensor.transpose(
                        pT_ps[64 * j:64 * j + M, h * M:(h + 1) * M],
                        prob[64 * j:64 * j + M, h * M:(h + 1) * M],
                        ident[64 * j:64 * j + M, 64 * j:64 * j + M])
            pT = ptpool.tile([128, H * M], bf16, tag="pT")
            nc.any.tensor_copy(pT[:], pT_ps[:])
            # ctx: per (h, j) matmul -> psum [128, 512]
            cx = pcx.tile([128, DIM], fp32, tag="cx")
            for h in range(H):
                for j in range(2):
                    nc.tensor.matmul(
                        cx[64 * j:64 * j + M, h * HD:(h + 1) * HD],
                        pT[64 * j:64 * j + M, h * M:(h + 1) * M],
                        v1[64 * j:64 * j + M,
                           g2 * DIM + h * HD: g2 * DIM + (h + 1) * HD],
                        start=True, stop=True)
            # ctx -> x2 [s-part, (m, o)] via SBUF->SBUF permute DMA
            cs = ctxpool.tile([128, DIM], bf16, tag="cs")
            nc.any.tensor_copy(cs[:], cx[:])
            for j in range(2):
                nc.sync.dma_start(x2[s0 + j:s0 + j + 1, :],
                                  cs[64 * j:64 * j + M, :])


# ---------------------------------------------------------------------------
# QKV2
# ---------------------------------------------------------------------------

def _qkv2(tc, nc, b, x2, w2, bqk_sb, bv2_sb, q2T_d, k2T_d, v2_d,
          ident, fp32, bf16, AF, ALU, tile):
    x2v = x2[:].rearrange("s (m o) -> s m o", m=M)
    with (
        tc.tile_pool(name="x2T", bufs=2) as xtpool,
        tc.tile_pool(name="v2s", bufs=3) as v2pool,
        tc.tile_pool(name="ps_tr2", bufs=3, space="PSUM") as ptr,
        tc.tile_pool(name="ps_qk2", bufs=3, space="PSUM") as pqk,
        tc.tile_pool(name="ps_v2", bufs=2, space="PSUM") as pv,
        tc.tile_pool(name="qkst", bufs=6) as qkstpool,
    ):
        for st in range(M // 4):  # supertiles of 4 m-tiles (512 tokens)
            ms = [4 * st + i for i in range(4)]
            xT = [xtpool.tile([128, 512], bf16, name=f"xT{ic}", tag=f"xT{ic}") for ic in range(4)]
            for ic in range(4):
                for mi, m in enumerate(ms):
                    tp = ptr.tile([128, 128], bf16, tag="tr")
                    nc.tensor.transpose(
                        tp[:], x2v[:, m, ic * 128:(ic + 1) * 128], ident[:])
                    nc.any.tensor_copy(xT[ic][:, mi * 128:(mi + 1) * 128], tp[:])
            for mi2, (mat, dstT, bcol) in enumerate(
                    (("Wq2", q2T_d, 2), ("Wk2", k2T_d, 3))):
                for oc in range(4):
                    acc = pqk.tile([128, 512], fp32, tag="qk")
                    for ic in range(4):
                        nc.tensor.matmul(
                            acc[:],
                            w2[mat][:, ic * DIM + oc * 128: ic * DIM + (oc + 1) * 128],
                            xT[ic][:], start=(ic == 0), stop=(ic == 3))
                    qkst = qkstpool.tile([128, 512], bf16, tag="qkst")
                    bias_ap = bqk_sb[:, bcol * 4 + oc: bcol * 4 + oc + 1]
                    if (oc + mi2) % 2 == 0:
                        nc.vector.tensor_scalar(
                            qkst[:], acc[:], bias_ap, None, op0=ALU.add)
                    else:
                        nc.scalar.activation(
                            qkst[:], acc[:], AF.Identity, bias=bias_ap)
                    nc.sync.dma_start(
                        dstT[b, oc][:, st * 512:(st + 1) * 512], qkst[:])
            # V2 (form A) per m-tile
            for mi, m in enumerate(ms):
                accv = pv.tile([128, DIM], fp32, tag="v")
                for ic in range(4):
                    nc.tensor.matmul(
                        accv[:], xT[ic][:, mi * 128:(mi + 1) * 128],
                        w2["Wv2"][:, ic * DIM:(ic + 1) * DIM],
                        start=(ic == 0), stop=(ic == 3))
                vs = v2pool.tile([128, DIM], bf16, tag="vs")
                nc.vector.tensor_tensor(vs[:], accv[:], bv2_sb[:], ALU.add)
                nc.sync.dma_start(
                    v2_d[b, m].rearrange("s o -> s o"), vs[:])


# ---------------------------------------------------------------------------
# branch2 attention
# ---------------------------------------------------------------------------

def _branch2(tc, nc, b, q2T_d, k2T_d, v2_d, mb2_sb, rkT_d, rv_d, out_d,
             ident, identf, ones1, fp32, bf16, AF, ALU, AX, tile, dbg):
    HALF = M // 2  # 18
    NT = M * SEQ
    with (
        tc.tile_pool(name="b2big", bufs=1) as big,
        tc.tile_pool(name="b2stat", bufs=1) as statpool,
    ):
        q2T = [big.tile([128, NT], bf16, name=f"q2Ts{h}", tag=f"q2Ts{h}")
               for h in range(H)]
        k2T = [big.tile([128, NT], bf16, name=f"k2Ts{h}", tag=f"k2Ts{h}")
               for h in range(H)]
        for h in range(H):
            nc.sync.dma_start(q2T[h][:], q2T_d[b, h])
            nc.sync.dma_start(k2T[h][:], k2T_d[b, h])
        p2T = big.tile([128, M * H * 128], bf16, tag="p2T")
        c2T = [big.tile([128, M * 128], bf16, name=f"c2T{h}", tag=f"c2T{h}") for h in range(H)]
        rs_all = statpool.tile([128, M * H], fp32, tag="rs")

        for half in range(2):
            n0 = half * HALF
            # ---- rel-k pass for this half ----
            with tc.tile_pool(name="relk", bufs=1) as relpool:
                relT = relpool.tile([128, HALF * H * 128], fp32, tag="relT")
                with (
                    tc.tile_pool(name="rkt", bufs=6) as rktpool,
                    tc.tile_pool(name="ps_rel", bufs=4, space="PSUM") as prel,
                ):
                    for q in range(SEQ):
                        rkt = rktpool.tile([128, 128], bf16, tag="rkt")
                        nc.sync.dma_start(rkt[:], rkT_d[q])
                        rp = prel.tile([128, H * HALF], fp32, tag="rp")
                        for h in range(H):
                            nc.tensor.matmul(
                                rp[:, h * HALF:(h + 1) * HALF], rkt[:],
                                q2T[h][:].rearrange("d (n q) -> d n q", q=SEQ)
                                [:, n0:n0 + HALF, q],
                                start=True, stop=True)
                        # scatter [k, (h, nn)] -> relT [k, (nn, h, q)]
                        nc.any.tensor_copy(
                            relT[:].rearrange("k (n h q) -> k n h q",
                                              h=H, q=SEQ)[:, :, :, q],
                            rp[:].rearrange("k (h n) -> k n h", h=H))

                # ---- main pass for this half ----
                with (
                    tc.tile_pool(name="b2s", bufs=3) as spool,
                    tc.tile_pool(name="b2e", bufs=6) as epool,
                    tc.tile_pool(name="b2v", bufs=4) as vpool,
                    tc.tile_pool(name="b2nm", bufs=8) as nmpool,
                    tc.tile_pool(name="ps_sc2", bufs=3, space="PSUM") as psc,
                    tc.tile_pool(name="ps_pt2", bufs=2, space="PSUM") as ppt,
                    tc.tile_pool(name="ps_cx2", bufs=2, space="PSUM") as pcx,
                ):
                    for nn in range(HALF):
                        n = n0 + nn
                        sc_ps = psc.tile([128, H * 128], fp32, tag="sc")
                        for h in range(H):
                            sl = sc_ps[:, h * 128:(h + 1) * 128]
                            nc.tensor.matmul(
                                sl, q2T[h][:, n * 128:(n + 1) * 128],
                                k2T[h][:, n * 128:(n + 1) * 128],
                                start=True, stop=False)
                            if mb2_sb is not None:
                                nc.tensor.matmul(
                                    sl, ones1[:],
                                    mb2_sb[:, (b * M + n) * 128:
                                           (b * M + n + 1) * 128],
                                    start=False, stop=False)
                            nc.tensor.matmul(
                                sl,
                                relT[:, (nn * H + h) * 128:(nn * H + h + 1) * 128],
                                identf[:], is_transpose=True,
                                start=False, stop=True)
                        s_sb = spool.tile([128, H * 128], fp32, tag="s")
                        nc.any.tensor_copy(s_sb[:], sc_ps[:])
                        nmax = nmpool.tile([128, H], fp32, tag="nmax")
                        nc.vector.tensor_reduce(
                            nmax[:], s_sb[:].rearrange("p (h k) -> p h k", h=H),
                            AX.X, ALU.max, negate=True)
                        e_sb = epool.tile([128, H * 128], bf16, tag="e")
                        for h in range(H):
                            nc.scalar.activation(
                                e_sb[:, h * 128:(h + 1) * 128],
                                s_sb[:, h * 128:(h + 1) * 128], AF.Exp,
                                bias=nmax[:, h:h + 1],
                                accum_out=rs_all[:, n * H + h:n * H + h + 1])
                        # transpose E per h -> p2T; ctx main matmuls
                        v2t = vpool.tile([128, DIM], bf16, tag="v2t")
                        nc.sync.dma_start(v2t[:], v2_d[b, n])
                        tp = ppt.tile([128, 4 * 128], bf16, tag="pt")
                        cxp = pcx.tile([128, 4 * 128], fp32, tag="cx")
                        for h in range(H):
                            nc.tensor.transpose(
                                tp[:, h * 128:(h + 1) * 128],
                                e_sb[:, h * 128:(h + 1) * 128], ident[:])
                            pslice = p2T[:, (n * H + h) * 128:(n * H + h + 1) * 128]
                            nc.any.tensor_copy(pslice, tp[:, h * 128:(h + 1) * 128])
                            nc.tensor.matmul(
                                cxp[:, h * 128:(h + 1) * 128],
                                v2t[:, h * HD:(h + 1) * HD], pslice,
                                start=True, stop=True)
                            nc.any.tensor_copy(
                                c2T[h][:, n * 128:(n + 1) * 128],
                                cxp[:, h * 128:(h + 1) * 128])

        # ---- rel-v pass (full batch) ----
        with (
            tc.tile_pool(name="rvt", bufs=6) as rvtpool,
            tc.tile_pool(name="ps_rv", bufs=4, space="PSUM") as prv,
        ):
            p2Tv = p2T[:].rearrange("k (n h q) -> k n h q", h=H, q=SEQ)
            for q in range(SEQ):
                rvt = rvtpool.tile([128, 128], bf16, tag="rvt")
                nc.sync.dma_start(rvt[:], rv_d[q])
                for h in range(H):
                    rp = prv.tile([128, M], fp32, tag="rp")
                    nc.tensor.matmul(rp[:], rvt[:], p2Tv[:, :, h, q],
                                     start=True, stop=True)
                    dst = c2T[h][:].rearrange("d (n q) -> d n q", q=SEQ)[:, :, q]
                    nc.vector.tensor_tensor(dst, rp[:], dst, ALU.add)

        # ---- output: transpose + 1/rowsum ----
        with (
            tc.tile_pool(name="ostat", bufs=1) as ostat,
            tc.tile_pool(name="osb", bufs=3) as opool,
            tc.tile_pool(name="ps_out", bufs=2, space="PSUM") as pout,
        ):
            rinv = ostat.tile([128, M * H], fp32, tag="rinv")
            nc.vector.reciprocal(rinv[:], rs_all[:])
            for n in range(M):
                op = pout.tile([128, DIM], bf16, tag="op")
                for h in range(H):
                    nc.tensor.transpose(
                        op[:, h * HD:(h + 1) * HD],
                        c2T[h][:, n * 128:(n + 1) * 128], ident[:])
                osb = opool.tile([128, DIM], fp32, tag="osb")
                for h in range(H):
                    nc.vector.tensor_scalar_mul(
                        osb[:, h * HD:(h + 1) * HD], op[:, h * HD:(h + 1) * HD],
                        rinv[:, n * H + h:n * H + h + 1])
                nc.sync.dma_start(out_d[b, n], osb[:])


# ---------------------------------------------------------------------------
# host side
# ---------------------------------------------------------------------------

_CACHE = {}


def _host_prep(hidden_states, attention_mask, sim_graph,
               Wq_sim, bq_sim, Wk_sim, bk_sim, Wv_sim, bv_sim,
               Wq_seq, bq_seq, Wk_seq, bk_seq, Wv_seq, bv_seq,
               rel_k, rel_v):
    """Build the 8 per-core input maps."""
    f32 = np.float32
    hs = np.asarray(hidden_states, f32)
    mask = np.asarray(attention_mask, f32)
    sg = np.asarray(sim_graph, f32)

    # branch1 graph bias, host-folded mask, pre-scaled by 1e4
    # sg_eff = where(mask_sim==0, 0, sg); bias = 1e4*sg_eff  (const dropped)
    mask_sim = mask.transpose(0, 2, 1).reshape(B * SEQ, M)  # [n, k]
    sg_eff = np.where(mask_sim[:, None, None, :] == 0, 0.0, sg) * 1e4
    sg_eff = sg_eff.reshape(B, SEQ, H, M, M).astype(f32)

    # branch2 mask bias rows
    mb2 = ((1.0 - mask.reshape(B * M, SEQ)) * -10000.0).astype(BF)
    mb2 = mb2.reshape(B, M * SEQ)

    d = _dist()
    rk_full = np.asarray(rel_k, f32)[d]          # [q, k, hd]
    rv_full = np.asarray(rel_v, f32)[d]          # [q, k, hd]
    rkT = np.ascontiguousarray(rk_full.transpose(0, 2, 1)).astype(BF)
    rv_b = np.ascontiguousarray(rv_full).astype(BF)

    ws = {"Wq1": np.asarray(Wq_sim, f32) * SC, "Wk1": Wk_sim,
          "Wv1": Wv_sim, "Wq2": np.asarray(Wq_seq, f32) * SC,
          "Wk2": Wk_seq, "Wv2": Wv_seq}
    ws = {k: np.asarray(v, f32).astype(BF) for k, v in ws.items()}
    bqk = np.stack([np.asarray(bq_sim, f32) * SC, np.asarray(bk_sim, f32),
                    np.asarray(bq_seq, f32) * SC, np.asarray(bk_seq, f32)])
    bv = np.stack([np.broadcast_to(np.asarray(bv_sim, f32), (128, DIM)),
                   np.broadcast_to(np.asarray(bv_seq, f32), (128, DIM))])
    bv = np.ascontiguousarray(bv)

    hs_bf = hs.astype(BF)
    in_maps = []
    for c in range(N_CORES):
        bs = slice(c * BSH, (c + 1) * BSH)
        in_maps.append(dict(
            hs=np.ascontiguousarray(hs_bf[bs]),
            sg=np.ascontiguousarray(sg_eff[bs]),
            mb2=np.ascontiguousarray(mb2[bs]),
            bqk=bqk, bvr=bv, rkT=rkT, rv=rv_b,
            **ws))
    return in_maps


def kernel(hidden_states, attention_mask, sim_graph,
           Wq_sim, bq_sim, Wk_sim, bk_sim, Wv_sim, bv_sim,
           Wq_seq, bq_seq, Wk_seq, bk_seq, Wv_seq, bv_seq,
           rel_k, rel_v, b=None, m=None, seq=None, dim=None, **_):
    from concourse.bass_utils import run_bass_kernel_spmd

    in_maps = _host_prep(hidden_states, attention_mask, sim_graph,
                         Wq_sim, bq_sim, Wk_sim, bk_sim, Wv_sim, bv_sim,
                         Wq_seq, bq_seq, Wk_seq, bk_seq, Wv_seq, bv_seq,
                         rel_k, rel_v)
    masked = not bool(np.all(np.asarray(attention_mask) == 1.0))
    key = ("nc", masked)
    if key not in _CACHE:
        _CACHE[key] = build_kernel(with_mask_bias=masked)
    res = run_bass_kernel_spmd(_CACHE[key], in_maps,
                               core_ids=list(range(N_CORES)))
    outs = [r["out"] for r in res.results]
    return np.concatenate(outs, axis=0)
